# revision 4
# baseline (speedup 1.0000x reference)
"""LIF spiking-neuron scan (SimpleSNN) Trainium2 Bass kernel.

Reference semantics (per sample b, neuron n, over T=200 timesteps):
    mem = mem * 0.9 + x[t]
    spike[t] = (mem >= 1.5)
    mem = mem * (1 - spike[t])

Full inputs [256, 200, 1024] f32 are sharded batch-wise over 8 NeuronCores
(32 samples/core; the time recurrence is per-sample so no cross-core comms).

Host-side, each core's shard [32, 200, 1024] is permuted to a
partition-major layout [128, 200, 256] with partition p = k*32 + b
(k = n // 256, b = sample), so every chunk DMA is a single dense 3-D
transfer carrying one completion semaphore.

Per-core device design (all compute on the Vector engine):
  - The recurrence is rewritten over the PRE-reset membrane w:
        w_t = select(w_{t-1} < 1.5, w_{t-1}, 0) * 0.9 + x_t
        spike_t = (w_t >= 1.5)
    which is bit-identical to the reference (same two f32 roundings per
    step) and needs only ONE fused custom-DVE op per step. The w history
    is materialized in the chunk tile, so the whole sequential chain is
    200 back-to-back Vector-engine instructions at [128, 256] each.
  - T=200 steps split into chunks (small first/last chunks for pipeline
    ramp/tail). Per chunk: one HWDGE DMA load of x [128, tc, 256] (SP
    ring), tc fused LIF-step ops (DVE), one batched stock tensor_scalar
    is_ge over the w chunk producing bf16 {0,1} spikes (DVE, 2x mode),
    one HWDGE DMA store of the bf16 spike chunk (ACT ring). GpSimd only
    does the one-time zero memset (its stock tensor_scalar measured
    ~13 cyc/elem on HW — ~70 us per chunk — which was the old baseline's
    dominant cost).
  - Spikes travel to HBM as bf16 (exact for 0.0/1.0) halving store
    traffic; the host upcasts to f32 during unshard.

HW notes (measured via in-NEFF-repetition benches on the axon-tunneled
TRN2 cores; NTFF profiling is unavailable in this container):
  - Per-instruction dispatch costs ~2.8-3.3 us on EVERY engine (DVE, ACT,
    GpSimd alike), independent of op kind, tensor size (FD 64..6400), and
    data dependencies. Kernel wall time ~= busiest-engine instruction
    count x that tax; the DVE stream (200 chain + 10 spike ops) binds.
    DMA bytes ride entirely under it.
  - The LIF chain cannot be shortened: reset-LIF has no associative-scan
    or multi-step-per-instruction form (custom DVE ops are 2-stream, 1:1
    rate, feed-forward; tensor_tensor_scan folds through a single ALU op).
  - Tile's per-op self-semaphore waits (DVE_x >= k on every DVE op) are
    stripped post-compile (program order + the unconditional inter-op
    pipeline DRAIN already serialize same-engine ops); measured neutral
    on HW but free, and required headroom for quiet-device sessions.
  - Paired same-session A/B, 12 reps in-NEFF, best-of batches:
    this kernel ~700 us/rep vs previous baseline ~1410 us/rep (2.02x).
"""

from contextlib import ExitStack

import numpy as np

B, T, N = 256, 200, 1024
NCORES = 8
BL = B // NCORES  # 32 samples per core
DECAY = 0.9
TH = 1.5
P128 = 128
FREE = 256
NK = N // FREE  # 4 n-blocks; partition p = k*32 + b
CHUNKS = [13] + [25] * 7 + [12]

_CACHE = {}

_LIF_OP_NAME = "LIF_STEP_ANT"


def _lif_reference(in0, in1, s0, s1, imm2):
    return (
        np.where(in0 < np.float32(s0), in0, np.float32(0.0)) * np.float32(s1) + in1
    ).astype(np.float32)


def _register_lif_op():
    """Register the fused LIF-step custom DVE op:
        out = select(in0 < s0, in0, 0) * s1 + in1
    (in0 = previous membrane w, in1 = x_t, s0 = threshold, s1 = decay).
    """
    import concourse.dve_ops as dve_ops
    from concourse.dve_ops import DveOp
    from concourse.dve_spec import C0, C1, Spec, Src0, Src1, Zero, lower, select
    from concourse.dve_uop import DveOpSpec

    if _LIF_OP_NAME in dve_ops._SUB_OPCODE_FOR_NAME:
        for op in dve_ops.OPS:
            if op.name == _LIF_OP_NAME:
                return op
        raise RuntimeError("LIF op registered but not in OPS")

    body = select(Src0 < C0, Src0, Zero) * C1 + Src1
    spec = Spec(body=body, reference=_lif_reference)
    row = dve_ops._CUSTOM_DVE_ROW_BASE + len(dve_ops.OPS)
    shas = {}
    for ver in ("v3", "v4"):
        uops = lower(spec, ver=ver)
        shas[ver] = DveOpSpec(
            name=_LIF_OP_NAME, opcode=row, uops=uops, rd1_en=True
        ).sha(ver)
    op = DveOp(_LIF_OP_NAME, spec, subdim=False, uops_sha=shas)
    dve_ops.OPS.append(op)
    dve_ops._SUB_OPCODE_FOR_NAME[_LIF_OP_NAME] = row
    dve_ops.CUSTOM_DVE_SPECS[_LIF_OP_NAME] = spec
    return op


def _build_bass(chunks=None, xp_bufs=2, wp_bufs=2, sp_bufs=2, reps=1):
    # reps > 1 repeats the whole pipeline on the same buffers (benchmarking
    # only — amortizes per-call overhead to expose steady-state device time).
    import concourse.bacc as bacc
    import concourse.tile as tile
    from concourse import mybir

    lif_op = _register_lif_op()

    nc = bacc.Bacc("TRN2", target_bir_lowering=False, debug=False,
                   enable_asserts=False)

    f32 = mybir.dt.float32
    bf16 = mybir.dt.bfloat16

    x_d = nc.dram_tensor("x", [P128, T, FREE], f32, kind="ExternalInput").ap()
    s_d = nc.dram_tensor("spk", [P128, T, FREE], bf16, kind="ExternalOutput").ap()

    chunks = chunks or CHUNKS
    tcmax = max(chunks)

    with ExitStack() as ctx:
        tc = ctx.enter_context(tile.TileContext(nc))
        xp = ctx.enter_context(tc.tile_pool(name="xp", bufs=xp_bufs))
        wp = ctx.enter_context(tc.tile_pool(name="wp", bufs=wp_bufs))
        sp = ctx.enter_context(tc.tile_pool(name="sp", bufs=sp_bufs))
        st = ctx.enter_context(tc.tile_pool(name="st", bufs=1))

        zero = st.tile([P128, FREE], f32)
        # on GpSimd: keeps the DVE stream at exactly the 200 chain ops + 10
        # spike ops (per-instruction dispatch is the binding cost on HW)
        nc.gpsimd.memset(zero[:], 0.0)

        wt_prev = None
        prev_tc = None
        for c, tcsz in enumerate(chunks * reps):
            t0 = sum(chunks[: c % len(chunks)])
            xt = xp.tile([P128, tcmax, FREE], f32, tag="x")
            nc.sync.dma_start(out=xt[:, :tcsz, :], in_=x_d[:, t0 : t0 + tcsz, :])

            wt = wp.tile([P128, tcmax, FREE], f32, tag="w")
            for j in range(tcsz):
                if c == 0 and j == 0:
                    w_in = zero[:]
                elif j == 0:
                    w_in = wt_prev[:, prev_tc - 1, :]
                else:
                    w_in = wt[:, j - 1, :]
                nc.vector._custom_dve(
                    lif_op, out=wt[:, j, :], in0=w_in, in1=xt[:, j, :],
                    s0=TH, s1=DECAY,
                )
            wt_prev = wt
            prev_tc = tcsz

            spt = sp.tile([P128, tcmax, FREE], bf16, tag="s")
            nc.vector.tensor_scalar(
                out=spt[:, :tcsz, :].rearrange("p t f -> p (t f)"),
                in0=wt[:, :tcsz, :].rearrange("p t f -> p (t f)"),
                scalar1=TH, scalar2=None, op0=mybir.AluOpType.is_ge,
            )
            nc.scalar.dma_start(out=s_d[:, t0 : t0 + tcsz, :], in_=spt[:, :tcsz, :])

    nc.compile()
    _strip_same_engine_waits(nc)
    return nc


def _strip_same_engine_waits(nc):
    """Remove per-instruction self-semaphore waits that are already
    guaranteed by same-engine program order.

    Tile's scheduler emits, for every engine instruction, a wait on the
    engine's own tick semaphore (e.g. DVE op k waits DVE_x >= k-1) plus an
    increment. On the in-order DVE the inter-op pipeline DRAIN already
    serializes consecutive ops (output-hazard barrier, unconditional), so a
    wait on the engine's OWN semaphore whose target value is covered by the
    increments of PRECEDING same-engine instructions is redundant — but on
    real HW it costs ~1-2.5 us per instruction in sequencer/sync-block
    handshake. Stripping them takes the 200-op LIF chain from ~2.9 us/op to
    near the architectural op cost. Cross-engine waits (DMA completion,
    buffer recycling, barriers) are untouched; all increments are kept so
    other engines' waits still fire.
    """
    from collections import defaultdict

    from concourse import mybir

    prefix = {
        "DVE": "DVE_",
        "Activation": "ACT_",
        "SP": "SP_",
        "Pool": "POOL_",
        "PE": "PE_",
    }
    incs = defaultdict(int)  # (engine, sem_id) -> cumulative incs
    stripped = kept = 0
    for bb in nc.m.functions[0].blocks:
        for ins in bb.instructions:
            si = ins.sync_info
            if si is None:
                continue
            eng = str(ins.engine).split(".")[-1]
            pref = prefix.get(eng)
            new_waits = []
            changed = False
            for w in si.on_wait:
                if (
                    pref is not None
                    and w.sync_type == "semaphore"
                    and w.wait_mode == "sem-ge-imm"
                    and (w.ant_name or "").startswith(pref)
                    and incs[(eng, w.id)] >= w.wait_value
                ):
                    stripped += 1
                    changed = True
                    continue
                new_waits.append(w)
                kept += 1
            for u in si.on_update:
                if u.update_mode == "sem-inc":
                    incs[(eng, u.id)] += u.update_value
                elif u.update_mode == "sem-add-imm":
                    incs[(eng, u.id)] += u.update_value
            if changed:
                ins.sync_info = mybir.SyncInfo(
                    on_wait=new_waits, on_update=list(si.on_update)
                )
    return stripped, kept


def _get_nc():
    if "nc" not in _CACHE:
        _CACHE["nc"] = _build_bass()
    return _CACHE["nc"]


def _shard_input(inputs: np.ndarray, i: int) -> np.ndarray:
    # [32, 200, 1024] -> [32, 200, 4, 256] -> [4, 32, 200, 256] -> [128, 200, 256]
    xi = inputs[i * BL : (i + 1) * BL]
    xi = xi.reshape(BL, T, NK, FREE).transpose(2, 0, 1, 3)
    return np.ascontiguousarray(xi).reshape(P128, T, FREE)


def _unshard_output(spk: np.ndarray) -> np.ndarray:
    # [128, 200, 256] bf16 -> [4, 32, 200, 256] -> [32, 200, 4, 256] -> f32 [32, 200, 1024]
    s = spk.reshape(NK, BL, T, FREE).transpose(1, 2, 0, 3)
    return np.ascontiguousarray(s).astype(np.float32).reshape(BL, T, N)


def kernel(inputs: np.ndarray, trace: bool = False) -> np.ndarray:
    from concourse.bass_utils import run_bass_kernel_spmd

    inputs = np.ascontiguousarray(np.asarray(inputs, dtype=np.float32))
    assert inputs.shape == (B, T, N), inputs.shape

    nc = _get_nc()
    in_maps = [{"x": _shard_input(inputs, i)} for i in range(NCORES)]
    res = run_bass_kernel_spmd(
        nc, in_maps, core_ids=list(range(NCORES)), trace=trace
    )
    _CACHE["last_results"] = res
    out = np.concatenate(
        [_unshard_output(r["spk"]) for r in res.results], axis=0
    )
    return out
